# revision 9
# baseline (speedup 1.0000x reference)
"""CRF loss kernel for Trainium2 (8 NeuronCores, Bass/Tile).

Strategy
--------
Forward algorithm (log partition) in the exp domain: with E = exp(transitions)
shared across all timesteps, the recurrence is p <- diag(exp(emit_t)) @ E @ p.
Because E is shared, many sub-sequences can be scanned in parallel as columns
of a single [16, S] state driven by one PE matmul + one DVE columnwise mul per
step.  Each of the 8 cores owns a 4096-step chunk split into 128 sub-chunks of
L=32 steps; every sub-chunk starts B=16 steps early ("burn-in") from an
arbitrary positive state -- the Perron-Frobenius contraction of random positive
matrices makes the state direction converge to the true one in <16 steps (f32
bit-exact by ~20).  Per-column magnitudes are kept in range by a blind 2^-42
rescale every 8 steps; exact log-scale accounting happens only at the chunk
boundaries via column sums (alpha at chunk start, beta at chunk end):

    log rho_s = k*42*ln2 + ln(beta_s) - ln(alpha_s)      (k = 3 rescales)
    logZ      = sum_s log rho_s + ln(u . v_end) ,  u = exp(trans[STOP,:])

Gold path score: one-hot tags built with a single tensor_tensor is_equal
(broadcast APs), then PE-accumulated pair-count matrix C = O^T @ O_prev and
emission matrix D2 = O^T @ F;  gold = <C, trans> + trace(D2).

The host only shards inputs, sums 8 per-core scalars and applies two boundary
terms.
"""

import math

import numpy as np

import concourse.bacc as bacc
import concourse.bass as bass
import concourse.tile as tile
from concourse import mybir
from concourse.bass_utils import run_bass_kernel_spmd
from concourse.masks import make_identity

# ---- problem constants (hardcoded per contract) ----
T = 32768
K = 16
NC = 8
TC = T // NC          # 4096 timesteps per core
S = 128               # sub-chunks (columns) per core
L = TC // S           # 32 real steps per column
B = 16                # burn-in steps
STEPS = B + L         # 48
WWIN = STEPS          # per-partition window rows (48 = 6 blocks of 8)
NBLK = WWIN * 2 * K // 128   # 12 transpose blocks of [128,128]
RS_EVERY = 8
RS_L2 = 42            # rescale factor 2^-42
N_RESC = 3            # rescales strictly inside (alpha, beta] window: tau=24,32,40
START = 14
STOP = 15
NST = 2               # independent pipeline streams
SP = S // NST
FDT = mybir.dt.float32

_CACHE: dict = {}


def _build_kernel():
    nc = bacc.Bacc("TRN2", target_bir_lowering=False, debug=False, num_devices=NC)

    featsw = nc.dram_tensor("featsw", [(S - 1) * L + WWIN, K], FDT,
                            kind="ExternalInput").ap()      # [4112, 16]
    tagsw = nc.dram_tensor("tagsw", [TC + 1], FDT, kind="ExternalInput").ap()
    transT = nc.dram_tensor("transT", [K, K], FDT, kind="ExternalInput").ap()
    trans = nc.dram_tensor("trans", [K, K], FDT, kind="ExternalInput").ap()
    initmv = nc.dram_tensor("initmv", [K, 2], FDT, kind="ExternalInput").ap()
    out = nc.dram_tensor("out", [1, 8], FDT, kind="ExternalOutput").ap()

    with tile.TileContext(nc) as tc:
        with (
            tc.tile_pool(name="singles", bufs=1) as singles,
            tc.tile_pool(name="qps", bufs=2, space="PSUM") as qps,
            tc.tile_pool(name="tps", bufs=2, space="PSUM") as tps,
            tc.tile_pool(name="gps", bufs=1, space="PSUM") as gps,
            tc.tile_pool(name="sps", bufs=2, space="PSUM") as sps,
        ):
            # ---------------- loads ----------------
            raww = singles.tile([128, WWIN, K], FDT)      # scan window, overlapped
            nc.sync.dma_start(
                out=raww,
                in_=bass.AP(tensor=featsw.tensor, offset=0,
                            ap=[[L * K, 128], [K, WWIN], [1, K]]),
            )
            goldf = singles.tile([128, L, K], FDT)        # core's own 4096 rows
            nc.sync.dma_start(
                out=goldf,
                in_=bass.AP(tensor=featsw.tensor, offset=B * K,
                            ap=[[L * K, 128], [K, L], [1, K]]),
            )
            tsb = singles.tile([128, L + 1], FDT)         # tags window, overlap 1
            nc.sync.dma_start(
                out=tsb,
                in_=bass.AP(tensor=tagsw.tensor, offset=0,
                            ap=[[L, 128], [1, L + 1]]),
            )
            transT_sb = singles.tile([K, K], FDT)
            nc.sync.dma_start(out=transT_sb, in_=transT)
            trans_sb = singles.tile([K, K], FDT)
            nc.sync.dma_start(out=trans_sb, in_=trans)
            initmv_sb = singles.tile([K, 2], FDT)
            nc.sync.dma_start(out=initmv_sb, in_=initmv)

            # ---------------- constants ----------------
            ident128 = singles.tile([128, 128], FDT)
            make_identity(nc, ident128)
            ones16 = singles.tile([K, 1], FDT)
            nc.vector.memset(ones16, 1.0)
            ident16 = singles.tile([K, K], FDT)
            make_identity(nc, ident16)
            iota_i = singles.tile([128, K], mybir.dt.int32)
            nc.gpsimd.iota(iota_i, pattern=[[1, K]], base=0, channel_multiplier=0)
            iota_f = singles.tile([128, K], FDT)
            nc.vector.tensor_copy(iota_f, iota_i)

            # ---------------- exp + transpose into scan layout ----------------
            # E^T[k,i] = exp(trans[i,k]);  scan matmul lhsT = E^T.
            ET = singles.tile([K, K], FDT)
            nc.scalar.activation(ET, transT_sb, mybir.ActivationFunctionType.Exp)
            # exp with a 32-wide stride per timestep so that after the [128,128]
            # transposes every timestep's 16 rows start at a 32-aligned
            # partition (DVE requires 32-aligned partition starts).
            expw = singles.tile([128, WWIN, 2 * K], FDT)
            nc.vector.memset(expw, 0.0)
            nc.scalar.activation(expw[:, :, 0:K], raww,
                                 mybir.ActivationFunctionType.Exp)
            # dexp[(w%4)*32 + i, (w//4)*128 + s] = exp(feats[base + 32*s + w, i])
            dexp = singles.tile([128, NBLK * 128], FDT)
            expwf = expw.rearrange("p w i -> p (w i)")
            for b in range(NBLK):
                pst = tps.tile([128, 128], FDT)
                nc.tensor.transpose(pst, expwf[:, b * 128:(b + 1) * 128], ident128)
                nc.vector.tensor_copy(dexp[:, b * 128:(b + 1) * 128], pst)

            # ---------------- scan ----------------
            P = singles.tile([K, S], FDT)
            nc.vector.memset(P, 1.0)
            alpha_ps = sps.tile([1, S], FDT, tag="sp")
            lnab = singles.tile([1, 2 * S], FDT)

            rs_const = float(2.0 ** (-RS_L2))
            for tau in range(STEPS):
                for g in range(NST):
                    Pg = P[:, g * SP:(g + 1) * SP]
                    if tau > 0 and tau % RS_EVERY == 0:
                        nc.vector.tensor_scalar_mul(Pg, Pg, rs_const)
                    if tau == B:
                        if g == 0:
                            # core 0 only (via per-core mask/value inputs):
                            # P[:,0] = P[:,0]*mask + val  -> exact e_START
                            nc.vector.tensor_tensor(
                                P[:, 0:1], P[:, 0:1], initmv_sb[:, 0:1],
                                mybir.AluOpType.mult)
                            nc.vector.tensor_add(P[:, 0:1], P[:, 0:1],
                                                 initmv_sb[:, 1:2])
                        nc.tensor.matmul(alpha_ps[:, g * SP:(g + 1) * SP],
                                         ones16, Pg, start=True, stop=True)
                        if g == NST - 1:
                            nc.vector.tensor_copy(lnab[:, 0:S], alpha_ps)
                    q = qps.tile([K, SP], FDT, tag="q")
                    nc.tensor.matmul(q, ET, Pg, start=True, stop=True)
                    dsl = dexp[32 * (tau % 4):32 * (tau % 4) + K,
                               (tau // 4) * 128 + g * SP:(tau // 4) * 128 + (g + 1) * SP]
                    nc.vector.tensor_tensor(Pg, q, dsl, mybir.AluOpType.mult)

            beta_ps = sps.tile([1, S], FDT, tag="sp")
            for g in range(NST):
                nc.tensor.matmul(beta_ps[:, g * SP:(g + 1) * SP], ones16,
                                 P[:, g * SP:(g + 1) * SP], start=True, stop=True)
            nc.vector.tensor_copy(lnab[:, S:2 * S], beta_ps)

            # ---------------- epilogue: logs and fwd partial ----------------
            ln_out = singles.tile([1, 2 * S], FDT)
            sa = singles.tile([1, 1], FDT)
            sb2 = singles.tile([1, 1], FDT)
            nc.scalar.activation(ln_out[:, 0:S], lnab[:, 0:S],
                                 mybir.ActivationFunctionType.Ln, accum_out=sa)
            nc.scalar.activation(ln_out[:, S:2 * S], lnab[:, S:2 * S],
                                 mybir.ActivationFunctionType.Ln, accum_out=sb2)
            fwdp = singles.tile([1, 1], FDT)
            nc.vector.tensor_sub(fwdp, sb2, sa)

            # u . P[:, last]  with u[k] = exp(trans[STOP, k]) = ET[:, STOP]
            ud_ps = sps.tile([1, 1], FDT, tag="sp")
            nc.tensor.matmul(ud_ps, ET[:, STOP:STOP + 1], P[:, S - 1:S],
                             start=True, stop=True)
            ln_ud = singles.tile([1, 1], FDT)
            nc.scalar.activation(ln_ud, ud_ps, mybir.ActivationFunctionType.Ln)

            # ---------------- gold ----------------
            O = singles.tile([128, L, K], FDT)
            Op = singles.tile([128, L, K], FDT)
            nc.vector.tensor_tensor(
                O, tsb[:, 1:L + 1].unsqueeze(2).broadcast_to([128, L, K]),
                iota_f.unsqueeze(1).broadcast_to([128, L, K]),
                mybir.AluOpType.is_equal)
            nc.vector.tensor_tensor(
                Op, tsb[:, 0:L].unsqueeze(2).broadcast_to([128, L, K]),
                iota_f.unsqueeze(1).broadcast_to([128, L, K]),
                mybir.AluOpType.is_equal)
            gc_ps = gps.tile([K, K], FDT)
            gd_ps = gps.tile([K, K], FDT)
            for w in range(L):
                st, sp = (w == 0), (w == L - 1)
                nc.tensor.matmul(gc_ps, O[:, w, :], Op[:, w, :], start=st, stop=sp)
                nc.tensor.matmul(gd_ps, O[:, w, :], goldf[:, w, :], start=st, stop=sp)
            tmp = singles.tile([K, 2 * K], FDT)
            acc = singles.tile([K, 1], FDT)
            nc.vector.tensor_tensor(tmp[:, 0:K], gc_ps, trans_sb,
                                    mybir.AluOpType.mult)
            nc.vector.tensor_tensor(tmp[:, K:2 * K], gd_ps, ident16,
                                    mybir.AluOpType.mult)
            nc.vector.reduce_sum(acc, tmp, axis=mybir.AxisListType.X)
            gp_ps = sps.tile([1, 1], FDT, tag="sp")
            nc.tensor.matmul(gp_ps, ones16, acc, start=True, stop=True)

            # ---------------- output ----------------
            osb = singles.tile([1, 8], FDT)
            nc.vector.memset(osb, 0.0)
            nc.vector.tensor_copy(osb[:, 0:1], fwdp)
            nc.vector.tensor_copy(osb[:, 1:2], ln_ud)
            nc.vector.tensor_copy(osb[:, 2:3], ln_out[:, 2 * S - 1:2 * S])
            nc.vector.tensor_copy(osb[:, 3:4], gp_ps)
            nc.sync.dma_start(out=out, in_=osb)

    nc.compile()
    return nc


def _get_nc():
    if "nc" not in _CACHE:
        _CACHE["nc"] = _build_kernel()
    return _CACHE["nc"]


def _make_in_maps(feats, tags, transitions):
    feats = np.ascontiguousarray(feats, dtype=np.float32)
    tags_i = np.asarray(tags).astype(np.int64)
    trans = np.ascontiguousarray(transitions, dtype=np.float32)

    featsP = np.vstack([np.zeros((B, K), np.float32), feats])
    tagsX = np.concatenate([[START], tags_i]).astype(np.float32)
    transT = np.ascontiguousarray(trans.T)

    in_maps = []
    for c in range(NC):
        base = c * TC
        mv = np.zeros((K, 2), np.float32)
        if c == 0:
            mv[START, 1] = 1.0       # mask=0, val=e_START
        else:
            mv[:, 0] = 1.0           # mask=1, val=0
        in_maps.append({
            "featsw": np.ascontiguousarray(featsP[base: base + (S - 1) * L + WWIN]),
            "tagsw": np.ascontiguousarray(tagsX[base: base + TC + 1]),
            "transT": transT,
            "trans": trans,
            "initmv": mv,
        })
    return in_maps, tags_i, trans


def _combine(outs, tags_i, trans):
    fwd = sum(float(o["out"][0, 0]) for o in outs)
    fwd += NC * S * N_RESC * RS_L2 * math.log(2.0)
    logZ = fwd + float(outs[-1]["out"][0, 1]) - float(outs[-1]["out"][0, 2])
    gold = sum(float(o["out"][0, 3]) for o in outs)
    gold += float(trans[STOP, tags_i[-1]])
    return np.float32((logZ - gold) / T)


def kernel(feats, tags, transitions):
    nc = _get_nc()
    in_maps, tags_i, trans = _make_in_maps(feats, tags, transitions)
    res = run_bass_kernel_spmd(nc, in_maps, core_ids=list(range(NC)))
    return _combine(res.results, tags_i, trans)


if __name__ == "__main__":
    d = np.load("/root/problem/inputs_only.npz")
    loss = kernel(d["feats"], d["tags"], d["transitions"])
    print("loss:", loss)


# revision 16
# speedup vs baseline: 1.8251x; 1.8251x over previous
"""CRF loss kernel for Trainium2 (8 NeuronCores, Bass/Tile) — v2 banded scan.

Forward algorithm in the exp domain: p <- diag(exp(emit_t)) @ E @ p with
E = exp(transitions) shared across timesteps.  v2 packs EIGHT groups of
sub-sequences into the 128 partitions (group g occupies partitions
[16g, 16g+16)) with a block-diagonal E — one PE matmul [K=128, N=cols] and one
full-width DVE multiply per step.  Each core runs 512 sub-chunks of L=8 steps
(+B=8 burn-in from an arbitrary positive state; Perron-Frobenius contraction
makes the direction converge in <8 steps to f32 rounding).  Log-scale
accounting happens only at chunk boundaries via column sums (alpha/beta):

    log rho_col = ln(beta) - ln(alpha);   logZ = sum + ln(u . v_end)

The per-step exp(emit) slices are produced by PE transposes straight into
PSUM (partition layout g*16+i, one [128,64] slice per step) and consumed
there by the DVE multiply — no eviction copies.

Gold path score: one-hot tags via a single tensor_tensor is_equal with
broadcast APs; pair-count and emission sums accumulate on PE as
[C | D2] = O^T @ [O_prev | F];  gold = <C, trans> + trace(D2).

Host work: shard inputs, build the block-diagonal transitions pattern, sum 8
per-core scalars, add two boundary terms.
"""

import math

import numpy as np

import concourse.bacc as bacc
import concourse.bass as bass
import concourse.tile as tile
from concourse import mybir
from concourse.bass_utils import run_bass_kernel_spmd
from concourse.masks import make_identity

# ---- problem constants (hardcoded per contract) ----
T = 32768
K = 16
NC = 8
TC = T // NC            # 4096 timesteps per core
G = 8                   # partition groups
SPG = 64                # sub-chunks per group -> 512 columns/core
COLS = G * SPG
L = TC // COLS          # 8 real steps per column
B = 8                   # burn-in steps
STEPS = B + L           # 16
WWIN = STEPS            # window rows per column (16)
NCHUNK = 4              # preamble pipeline chunks (4 w's each)
RS_L2 = 42              # rescale factor 2^-42 applied once at tau=B
START = 14
STOP = 15
NST = 2                 # scan streams (split over s')
SH = SPG // NST         # 32 columns per stream
FDT = mybir.dt.float32
BDT = mybir.dt.bfloat16
FWIN = (COLS - 1) * L + WWIN   # 4104 feats rows per core

_CACHE: dict = {}


def _build_kernel():
    nc = bacc.Bacc("TRN2", target_bir_lowering=False, debug=False, num_devices=NC)

    featsw = nc.dram_tensor("featsw", [FWIN, K], FDT, kind="ExternalInput").ap()
    tagsw = nc.dram_tensor("tagsw", [TC + 1], FDT, kind="ExternalInput").ap()
    transTB = nc.dram_tensor("transTB", [128, 128], FDT, kind="ExternalInput").ap()
    trid = nc.dram_tensor("trid", [K, 2 * K], FDT, kind="ExternalInput").ap()
    initmv = nc.dram_tensor("initmv", [128, 2], FDT, kind="ExternalInput").ap()
    out = nc.dram_tensor("out", [1, 8], FDT, kind="ExternalOutput").ap()

    with tile.TileContext(nc) as tc:
        with (
            tc.tile_pool(name="singles", bufs=1) as singles,
            tc.tile_pool(name="qps", bufs=2, space="PSUM") as qps,
            tc.tile_pool(name="dbp", bufs=1, space="PSUM") as dbp,
            tc.tile_pool(name="gps", bufs=1, space="PSUM") as gps,
            tc.tile_pool(name="sps", bufs=2, space="PSUM") as sps,
        ):
            # ---------------- small loads ----------------
            transTB_sb = singles.tile([128, 128], FDT)
            nc.sync.dma_start(out=transTB_sb, in_=transTB)
            trid_sb = singles.tile([K, 2 * K], FDT)
            nc.sync.dma_start(out=trid_sb, in_=trid)
            initmv_sb = singles.tile([128, 2], FDT)
            nc.sync.dma_start(out=initmv_sb, in_=initmv)
            initmv_b = singles.tile([128, 2], BDT)
            # ETB = exp(transTB): block-diagonal E^T stack, bf16 for 1-pass MMs.
            # First ACT op -> exp table load overlaps the big feats DMAs.
            ETB = singles.tile([128, 128], BDT)
            nc.scalar.activation(ETB, transTB_sb, mybir.ActivationFunctionType.Exp)

            # gold-side loads on the scalar-engine DMA queue (parallel to sync)
            tsb = singles.tile([128, 33], FDT)
            nc.scalar.dma_start(
                out=tsb,
                in_=bass.AP(tensor=tagsw.tensor, offset=0,
                            ap=[[32, 128], [1, 33]]),
            )
            OpF = singles.tile([128, 32, 2 * K], FDT)
            nc.scalar.dma_start(
                out=OpF[:, :, K:2 * K],
                in_=bass.AP(tensor=featsw.tensor, offset=B * K,
                            ap=[[32 * K, 128], [K, 32], [1, K]]),
            )

            # ---------------- constants ----------------
            ident128 = singles.tile([128, 128], FDT)
            make_identity(nc, ident128)
            ones16 = singles.tile([K, 1], FDT)
            nc.vector.memset(ones16, 1.0)
            ones8 = singles.tile([G, 1], FDT)
            nc.vector.memset(ones8, 1.0)
            iota16 = singles.tile([128, K], mybir.dt.int32)
            nc.gpsimd.iota(iota16, pattern=[[1, K]], base=0, channel_multiplier=0)
            iota16f = singles.tile([128, K], FDT)
            nc.vector.tensor_copy(iota16f, iota16)
            # gsel[k, g] = (k>>4 == g), bf16 (matmul dtype must match state)
            iota8 = singles.tile([128, G], mybir.dt.int32)
            nc.gpsimd.iota(iota8, pattern=[[1, G]], base=0, channel_multiplier=0)
            iotap = singles.tile([128, 1], mybir.dt.int32)
            nc.gpsimd.iota(iotap, pattern=[[0, 1]], base=0, channel_multiplier=1)
            iotap4 = singles.tile([128, 1], mybir.dt.int32)
            nc.vector.tensor_scalar(iotap4, iotap, 4, None,
                                    mybir.AluOpType.logical_shift_right)
            iotap4f = singles.tile([128, 1], FDT)
            nc.vector.tensor_copy(iotap4f, iotap4)
            iota8f = singles.tile([128, G], FDT)
            nc.vector.tensor_copy(iota8f, iota8)
            gself = singles.tile([128, G], FDT)
            nc.vector.tensor_scalar(gself, iota8f, iotap4f, None,
                                    mybir.AluOpType.is_equal)
            gsel = singles.tile([128, G], BDT)
            nc.vector.tensor_copy(gsel, gself)
            # sel8[p,0] = (p == 7) on 8 partitions, f32
            iotap8 = singles.tile([G, 1], mybir.dt.int32)
            nc.gpsimd.iota(iotap8, pattern=[[0, 1]], base=0, channel_multiplier=1)
            iotap8f = singles.tile([G, 1], FDT)
            nc.vector.tensor_copy(iotap8f, iotap8)
            sel8 = singles.tile([G, 1], FDT)
            nc.vector.tensor_scalar(sel8, iotap8f, 7.0, None,
                                    mybir.AluOpType.is_equal)
            nc.vector.tensor_copy(initmv_b, initmv_sb)

            # ---------------- feats window: DMA -> exp -> transpose to PSUM ----
            # column c=(g*SPG+s') covers t in [base+c*L, +L); window rows
            # w in [0,16) map to featsw row c*L + w (base offset -B applied
            # on host via zero-padding).
            raww = singles.tile([SPG, G, WWIN, K], FDT)     # [64, 8, 16, 16]
            expw = singles.tile([SPG, WWIN, G, K], FDT)     # (g,i) contig per w
            dbt0 = dbp.tile([128, 8, SPG], FDT, tag="db0")
            dbt1 = dbp.tile([128, 8, SPG], FDT, tag="db1")
            dbt = [dbt0, dbt1]
            dbs = singles.tile([128, WWIN, SPG], FDT)
            CW = WWIN // NCHUNK                              # 4 w's per chunk
            for c in range(NCHUNK):
                nc.sync.dma_start(
                    out=raww[:, :, c * CW:(c + 1) * CW, :],
                    in_=bass.AP(tensor=featsw.tensor, offset=c * CW * K,
                                ap=[[L * K, SPG], [SPG * L * K, G], [K, CW],
                                    [1, K]]),
                )
                nc.scalar.activation(
                    expw[:, c * CW:(c + 1) * CW, :, :].transpose([0, 2, 1, 3]),
                    raww[:, :, c * CW:(c + 1) * CW, :],
                    mybir.ActivationFunctionType.Exp)
                for w in range(c * CW, (c + 1) * CW):
                    # [64, (g,i)=128] -> [128, 64] slice of PSUM D tile
                    nc.tensor.transpose(
                        dbt[w // 8][:, w % 8, :],
                        expw[:, w, :, :],
                        ident128[0:SPG, 0:SPG])
                nc.vector.tensor_copy(
                    dbs[:, c * CW:(c + 1) * CW, :],
                    dbt[(c * CW) // 8][:, (c * CW) % 8:(c * CW) % 8 + CW, :])

            # ---------------- scan ----------------
            Pb = singles.tile([128, SPG], BDT)
            nc.vector.memset(Pb, 1.0)
            asb = singles.tile([G, SPG], FDT)
            bsb = singles.tile([G, SPG], FDT)

            rs_const = float(2.0 ** (-RS_L2))
            for tau in range(STEPS):
                if tau == B:
                    nc.vector.tensor_scalar_mul(Pb, Pb, rs_const)
                    # core 0 only (mask/value inputs): column (g=0, s'=0)
                    nc.vector.tensor_tensor(Pb[:, 0:1], Pb[:, 0:1],
                                            initmv_b[:, 0:1],
                                            mybir.AluOpType.mult)
                    nc.vector.tensor_add(Pb[:, 0:1], Pb[:, 0:1],
                                         initmv_b[:, 1:2])
                    alpha_ps = sps.tile([G, SPG], FDT, tag="sp")
                    nc.tensor.matmul(alpha_ps, gsel, Pb, start=True, stop=True)
                    nc.vector.tensor_copy(asb, alpha_ps)
                for h in range(NST):
                    Ph = Pb[:, h * SH:(h + 1) * SH]
                    q = qps.tile([128, SH], FDT, tag="q")
                    nc.tensor.matmul(q, ETB, Ph, start=True, stop=True)
                    dsl = dbs[:, tau, h * SH:(h + 1) * SH]
                    nc.vector.tensor_tensor(Ph, q, dsl, mybir.AluOpType.mult)

            beta_ps = sps.tile([G, SPG], FDT, tag="sp")
            nc.tensor.matmul(beta_ps, gsel, Pb, start=True, stop=True)
            nc.vector.tensor_copy(bsb, beta_ps)

            # ---------------- gold (emitted after scan: fills engine gaps) ----
            O = singles.tile([128, 32, K], FDT)
            nc.vector.tensor_tensor(
                O, tsb[:, 1:33].unsqueeze(2).broadcast_to([128, 32, K]),
                iota16f.unsqueeze(1).broadcast_to([128, 32, K]),
                mybir.AluOpType.is_equal)
            nc.vector.tensor_tensor(
                OpF[:, :, 0:K],
                tsb[:, 0:32].unsqueeze(2).broadcast_to([128, 32, K]),
                iota16f.unsqueeze(1).broadcast_to([128, 32, K]),
                mybir.AluOpType.is_equal)
            g_ps = gps.tile([K, 2 * K], FDT)
            for w in range(32):
                nc.tensor.matmul(g_ps, O[:, w, :], OpF[:, w, :],
                                 start=(w == 0), stop=(w == 31))
            gtmp = singles.tile([K, 2 * K], FDT)
            gacc = singles.tile([K, 1], FDT)
            nc.vector.tensor_tensor(gtmp, g_ps, trid_sb, mybir.AluOpType.mult)
            nc.vector.reduce_sum(gacc, gtmp, axis=mybir.AxisListType.X)
            gp_ps = sps.tile([1, 1], FDT, tag="sp")
            nc.tensor.matmul(gp_ps, ones16, gacc, start=True, stop=True)

            # ---------------- epilogue ----------------
            ln_a = singles.tile([G, SPG], FDT)
            ln_b = singles.tile([G, SPG], FDT)
            sa = singles.tile([G, 1], FDT)
            sb2 = singles.tile([G, 1], FDT)
            nc.scalar.activation(ln_a, asb, mybir.ActivationFunctionType.Ln,
                                 accum_out=sa)
            nc.scalar.activation(ln_b, bsb, mybir.ActivationFunctionType.Ln,
                                 accum_out=sb2)
            d8 = singles.tile([G, 1], FDT)
            nc.vector.tensor_sub(d8, sb2, sa)
            fp_ps = sps.tile([1, 1], FDT, tag="sp")
            nc.tensor.matmul(fp_ps, ones8, d8, start=True, stop=True)

            # u . v_end: u = ETB[:, 127] (block g=7, row STOP); beta_last via sel8
            ud_ps = sps.tile([1, 1], FDT, tag="sp")
            nc.tensor.matmul(ud_ps, ETB[:, 127:128], Pb[:, SPG - 1:SPG],
                             start=True, stop=True)
            bl_ps = sps.tile([1, 1], FDT, tag="sp")
            nc.tensor.matmul(bl_ps, sel8, bsb[:, SPG - 1:SPG],
                             start=True, stop=True)
            ubl = singles.tile([1, 2], FDT)
            nc.vector.tensor_copy(ubl[:, 0:1], ud_ps)
            nc.vector.tensor_copy(ubl[:, 1:2], bl_ps)
            lnubl = singles.tile([1, 2], FDT)
            nc.scalar.activation(lnubl, ubl, mybir.ActivationFunctionType.Ln)

            osb = singles.tile([1, 8], FDT)
            nc.vector.memset(osb, 0.0)
            nc.vector.tensor_copy(osb[:, 0:1], fp_ps)
            nc.vector.tensor_copy(osb[:, 1:3], lnubl)
            nc.vector.tensor_copy(osb[:, 3:4], gp_ps)
            nc.sync.dma_start(out=out, in_=osb)

    nc.compile()
    return nc


def _get_nc():
    if "nc" not in _CACHE:
        _CACHE["nc"] = _build_kernel()
    return _CACHE["nc"]


def _make_in_maps(feats, tags, transitions):
    feats = np.ascontiguousarray(feats, dtype=np.float32)
    tags_i = np.asarray(tags).astype(np.int64)
    trans = np.ascontiguousarray(transitions, dtype=np.float32)

    featsP = np.vstack([np.zeros((B, K), np.float32), feats])
    tagsX = np.concatenate([[START], tags_i]).astype(np.float32)
    # block-diagonal transposed-transitions pattern; exp of -1e4 -> 0 off-blocks
    TB = np.full((128, 128), -10000.0, np.float32)
    for g in range(G):
        TB[g * K:(g + 1) * K, g * K:(g + 1) * K] = trans.T
    trid_v = np.ascontiguousarray(
        np.concatenate([trans, np.eye(K, dtype=np.float32)], axis=1))

    in_maps = []
    for c in range(NC):
        base = c * TC
        mv = np.zeros((128, 2), np.float32)
        mv[:, 0] = 1.0
        if c == 0:
            mv[0:K, 0] = 0.0
            mv[START, 1] = 1.0       # column (g=0, s'=0) -> e_START
        in_maps.append({
            "featsw": np.ascontiguousarray(featsP[base: base + FWIN]),
            "tagsw": np.ascontiguousarray(tagsX[base: base + TC + 1]),
            "transTB": TB,
            "trid": trid_v,
            "initmv": mv,
        })
    return in_maps, tags_i, trans


def _combine(outs, tags_i, trans):
    fwd = sum(float(o["out"][0, 0]) for o in outs)
    logZ = fwd + float(outs[-1]["out"][0, 1]) - float(outs[-1]["out"][0, 2])
    gold = sum(float(o["out"][0, 3]) for o in outs)
    gold += float(trans[STOP, tags_i[-1]])
    return np.float32((logZ - gold) / T)


def kernel(feats, tags, transitions):
    nc = _get_nc()
    in_maps, tags_i, trans = _make_in_maps(feats, tags, transitions)
    res = run_bass_kernel_spmd(nc, in_maps, core_ids=list(range(NC)))
    return _combine(res.results, tags_i, trans)


if __name__ == "__main__":
    d = np.load("/root/problem/inputs_only.npz")
    loss = kernel(d["feats"], d["tags"], d["transitions"])
    print("loss:", loss)
